# revision 21
# baseline (speedup 1.0000x reference)
"""Trainium2 Bass kernel for causal self-attention (B=2, T=2048, C=1024, H=16).

Sharding: tensor-parallel over heads. Each of the 8 cores owns 2 heads:
  - qkv weight column slices (128 q cols, 128 k cols, 128 v cols),
  - the full attention for its (batch, head) pairs,
  - a row-slice (128 rows) of w_proj -> partial [B*T, C] output.
Host side: x is transposed/cast to bf16 once (shared by all cores); the 8
partial proj outputs are summed + b_proj to form the final output.

Per-core device kernel (all matmuls bf16, fp32 accumulate):
  1. qkv^T projection: Q^T,K^T produced with head-dim on partitions
     ([128 = 2 heads x 64, T]); V produced in natural [t, v] layout with an
     appended ones column so the softmax denominator falls out of the AV
     matmul as a 65th output row.
  2. Causal attention over 128-wide k chunks x 512-wide q tiles computing
     S^T = [k, q] blocks for BOTH heads into one 2-bank PSUM tile
     [128, 2, 512] (PE, two matmuls on disjoint array row groups), one
     fused exp(scale*s) PSUM->SBUF bf16 per chunk (ACT), bf16 0/1
     triangle-mask multiply on the diagonal block (DVE), AV accumulation
     with the ones-row denominator (PE).
  3. Normalize: fast reciprocal of the denominator row (DVE),
     partition-broadcast (GPSIMD), multiply (DVE) -> attn_out^T bf16.
  4. proj emitted one q-tile late (PE never head-of-line blocks on the
     normalize chain); all phases share one PSUM pool so there are no
     pool-close barriers (a multi-us PE gap would re-throttle the PE clock
     to 1.2GHz via the HAM activity monitor).
"""

import numpy as np
import ml_dtypes
from contextlib import ExitStack

import concourse.bass as bass
import concourse.bacc as bacc
import concourse.mybir as mybir
import concourse.tile as tile

BF16 = mybir.dt.bfloat16
F32 = mybir.dt.float32

B, T, C = 2, 2048, 1024
H = 16
D = C // H  # 64
SCALE = D ** -0.5
N_CORES = 8
HEADS_PER_CORE = H // N_CORES  # 2
CL = HEADS_PER_CORE * D  # 128 local channels
P = 128
QT = 512  # q tile width (free dim of S^T blocks)
KC = 128  # k chunk (partition dim of S^T blocks)


def build_nc(t_batch=T, n_batch=B):
    """Build the per-core Bass module. t_batch/n_batch shrinkable for sim."""
    tt = t_batch * n_batch
    nqt = t_batch // QT  # q tiles per batch
    ntt = tt // QT       # t tiles for the qkv projection phase
    ntc = tt // P        # 128-wide t chunks
    cdim = C
    nco = cdim // P      # contraction chunks for qkv projection

    nc = bacc.Bacc("TRN2", target_bir_lowering=False)
    xt = nc.dram_tensor("xt", [cdim, tt], BF16, kind="ExternalInput")
    wq = nc.dram_tensor("wq", [cdim, 3 * CL], BF16, kind="ExternalInput")
    bq = nc.dram_tensor("bq", [3 * CL], F32, kind="ExternalInput")
    bv = nc.dram_tensor("bv", [P, CL], F32, kind="ExternalInput")
    wp = nc.dram_tensor("wp", [CL, C], BF16, kind="ExternalInput")
    mk = nc.dram_tensor("mk", [4 * P, QT], BF16, kind="ExternalInput")
    op = nc.dram_tensor("op", [tt, C], F32, kind="ExternalOutput")

    with tile.TileContext(nc) as tc, ExitStack() as ctx:
        singles = ctx.enter_context(tc.tile_pool(name="singles", bufs=1))
        # one shared PSUM pool for all phases: "s" = 2-bank [128, 2, 512]
        # S^T blocks (also qkv q/k psums and proj psums), o0/o1 = AV accums.
        psum = ctx.enter_context(tc.tile_pool(name="psum", bufs=2, space="PSUM"))
        xt_pool = ctx.enter_context(tc.tile_pool(name="xt_pool", bufs=2))
        pt_pool = ctx.enter_context(tc.tile_pool(name="pt_pool", bufs=3))
        nrm_pool = ctx.enter_context(tc.tile_pool(name="nrm_pool", bufs=2))
        ob_pool = ctx.enter_context(tc.tile_pool(name="ob_pool", bufs=3))

        qkT = singles.tile([P, 2, tt], BF16)  # [:,0,:]=Q^T  [:,1,:]=K^T
        vsb = singles.tile([P, ntc, 2 * (D + 1)], BF16)  # V_ext, both heads
        aoT = singles.tile([P, tt], BF16)  # normalized attn out^T
        wq_sb = singles.tile([P, nco, 3 * CL], BF16)
        bq_sb = singles.tile([P, 3], F32)
        bv_sb = singles.tile([P, CL], F32)
        wp_sb = singles.tile([CL, C], BF16)
        mask_sb = singles.tile([P, 4, QT], BF16)

        # Preloads go through the single SWDGE queue so downstream consumers
        # only inherit one DMA-semaphore wait.
        nc.gpsimd.dma_start(wq_sb, wq.rearrange("(co p) m -> p co m", p=P))
        nc.gpsimd.dma_start(bq_sb, bq.rearrange("(m p) -> p m", p=P))
        nc.gpsimd.dma_start(bv_sb, bv[:, :])
        nc.gpsimd.dma_start(wp_sb, wp[:, :])
        nc.gpsimd.dma_start(mask_sb, mk.rearrange("(mm p) q -> p mm q", p=P))

        # ---- Phase 1: qkv projection ----
        xt_r = xt.rearrange("(co p) t -> p co t", p=P)
        for j in range(ntt):
            ts = slice(j * QT, (j + 1) * QT)
            xt_t = xt_pool.tile([P, nco, QT], BF16, tag="xt")
            nc.sync.dma_start(xt_t, xt_r[:, :, ts])
            # Q^T and K^T: W chunk stationary, x^T moving; both halves of
            # one 2-bank psum tile, single fused bias-add copy-out.
            ps = psum.tile([P, 2, QT], F32, tag="s", name="ps_qk")
            for mi in range(2):
                for co in range(nco):
                    nc.tensor.matmul(
                        ps[:, mi, :],
                        lhsT=wq_sb[:, co, mi * CL:(mi + 1) * CL],
                        rhs=xt_t[:, co, :],
                        start=(co == 0), stop=(co == nco - 1),
                    )
            nc.vector.tensor_tensor(
                qkT[:, :, ts], ps,
                bq_sb[:, 0:2, None].to_broadcast((P, 2, QT)),
                op=mybir.AluOpType.add)
            # V natural: x^T chunk stationary, W_v moving
            for t2 in range(QT // P):
                tg = j * (QT // P) + t2
                psv = psum.tile([P, CL], F32, tag=f"o{t2 % 2}", name="ps_v")
                for co in range(nco):
                    nc.tensor.matmul(
                        psv,
                        lhsT=xt_t[:, co, t2 * P:(t2 + 1) * P],
                        rhs=wq_sb[:, co, 2 * CL:3 * CL],
                        start=(co == 0), stop=(co == nco - 1),
                    )
                for h in range(2):
                    o0 = h * (D + 1)
                    nc.vector.tensor_add(
                        vsb[:, tg, o0:o0 + D],
                        psv[:, h * D:(h + 1) * D], bv_sb[:, h * D:(h + 1) * D])
                    nc.vector.memset(vsb[:, tg, o0 + D:o0 + D + 1], 1.0)

        # ---- Phase 2+3: causal attention fused with output projection ----
        def emit_proj(base, j):
            # projection for q-tile j (emitted one q-tile late so the PE
            # never head-of-line blocks on the normalize chain)
            for t2 in range(QT // P):
                tg = (base + j * QT) // P + t2
                lhs = aoT[:, tg * P:(tg + 1) * P]
                pp = psum.tile([P, 2, QT], F32, tag="s", name="pp")
                for n in range(C // QT):
                    nc.tensor.matmul(pp[:, n, :], lhsT=lhs,
                                     rhs=wp_sb[:, n * QT:(n + 1) * QT],
                                     start=True, stop=True)
                ob = ob_pool.tile([P, C], F32, tag="ob")
                nc.vector.tensor_copy(ob, pp)
                nc.sync.dma_start(op[tg * P:(tg + 1) * P, :], ob)

        # big q-tiles first: the kernel tail (exp/normalize/proj after the
        # last QK matmul) is then the smallest tile's
        tile_order = [(b * t_batch, j) for j in reversed(range(nqt))
                      for b in range(n_batch)]
        pending = None
        for base, j in tile_order:
            if True:
                qs = slice(base + j * QT, base + (j + 1) * QT)
                os_ = [psum.tile([D + 1, QT], F32, tag=f"o{h}", name=f"o{h}")
                       for h in range(2)]
                nch = (j + 1) * (QT // KC)
                for m in range(nch):
                    ks = slice(base + m * KC, base + m * KC + KC)
                    mm = m - j * (QT // KC)
                    s = psum.tile([P, 2, QT], F32, tag="s", name="s")
                    for h in range(2):
                        hp = slice(64 * h, 64 * h + 64)
                        nc.tensor.matmul(s[:, h, :], lhsT=qkT[hp, 1, ks],
                                         rhs=qkT[hp, 0, qs],
                                         start=True, stop=True)
                    pt = pt_pool.tile([P, 2, QT], BF16, tag="pt")
                    if mm >= 0:
                        dcol = mm * KC  # diagonal block column offset
                        if dcol > 0:
                            nc.vector.memset(pt[:, :, 0:dcol], 0.0)
                        nc.scalar.activation(
                            pt[:, :, dcol:], s[:, :, dcol:],
                            mybir.ActivationFunctionType.Exp,
                            bias=0.0, scale=SCALE)
                        # zero the k>q triangle: bf16 0/1 multiply (SBUF)
                        for h in range(2):
                            nc.vector.tensor_mul(
                                pt[:, h, dcol:dcol + KC],
                                pt[:, h, dcol:dcol + KC],
                                mask_sb[:, mm, dcol:dcol + KC])
                    else:
                        nc.scalar.activation(
                            pt, s, mybir.ActivationFunctionType.Exp,
                            bias=0.0, scale=SCALE)
                    tg = (base + m * KC) // P
                    for h in range(2):
                        nc.tensor.matmul(
                            os_[h],
                            lhsT=vsb[:, tg, h * (D + 1):(h + 1) * (D + 1)],
                            rhs=pt[:, h, :],
                            start=(m == 0), stop=(m == nch - 1),
                        )
                # normalize: fast recip of denom row -> bcast (gpsimd) -> mul
                for h in range(2):
                    rec = nrm_pool.tile([1, QT], F32, tag="rec")
                    nc.vector.reciprocal_approx_fast(rec, os_[h][D:D + 1, :])
                    rbc = nrm_pool.tile([D, QT], F32, tag="rbc")
                    nc.gpsimd.partition_broadcast(rbc, rec)
                    nc.vector.tensor_mul(aoT[64 * h:64 * h + 64, qs],
                                         os_[h][0:D, :], rbc)
                if pending is not None:
                    emit_proj(*pending)
                pending = (base, j)
        if pending is not None:
            emit_proj(*pending)

    nc.finalize()
    return nc


def make_masks():
    """[4*128, 512] bf16 0/1 keep-masks: row kl of block mm, col c is kept
    iff global k (= 128*mm + kl) <= q (= c)."""
    kl = np.arange(P)[:, None]
    c = np.arange(QT)[None, :]
    out = np.zeros((4 * P, QT), ml_dtypes.bfloat16)
    for mm in range(4):
        out[mm * P:(mm + 1) * P] = (mm * P + kl <= c).astype(ml_dtypes.bfloat16)
    return out


def make_in_maps(x, w_qkv, b_qkv, w_proj, t_batch=T, n_batch=B):
    bf = ml_dtypes.bfloat16
    tt = t_batch * n_batch
    x2 = np.ascontiguousarray(x.reshape(tt, C))
    xt = np.ascontiguousarray(x2.T).astype(bf)
    masks = make_masks()
    in_maps = []
    for i in range(N_CORES):
        cs = slice(CL * i, CL * (i + 1))
        wq_c = np.concatenate(
            [w_qkv[:, cs], w_qkv[:, C + CL * i:C + CL * (i + 1)],
             w_qkv[:, 2 * C + CL * i:2 * C + CL * (i + 1)]], axis=1).astype(bf)
        bq_c = np.concatenate(
            [b_qkv[cs], b_qkv[C + CL * i:C + CL * (i + 1)],
             b_qkv[2 * C + CL * i:2 * C + CL * (i + 1)]]).astype(np.float32)
        bv_c = np.ascontiguousarray(np.broadcast_to(
            b_qkv[2 * C + CL * i:2 * C + CL * (i + 1)][None, :],
            (P, CL))).astype(np.float32)
        wp_c = np.ascontiguousarray(w_proj[cs, :]).astype(bf)
        in_maps.append({
            "xt": xt, "wq": np.ascontiguousarray(wq_c), "bq": bq_c,
            "bv": bv_c, "wp": wp_c, "mk": masks,
        })
    return in_maps


_CACHE = {}


def kernel(x, w_qkv, b_qkv, w_proj, b_proj):
    from concourse.bass_utils import run_bass_kernel_spmd

    x = np.asarray(x, np.float32)
    w_qkv = np.asarray(w_qkv, np.float32)
    b_qkv = np.asarray(b_qkv, np.float32)
    w_proj = np.asarray(w_proj, np.float32)
    b_proj = np.asarray(b_proj, np.float32)

    if "nc" not in _CACHE:
        _CACHE["nc"] = build_nc()
    nc = _CACHE["nc"]
    in_maps = make_in_maps(x, w_qkv, b_qkv, w_proj)
    res = run_bass_kernel_spmd(nc, in_maps, core_ids=list(range(N_CORES)))
    partial = np.zeros((B * T, C), np.float64)
    for r in res.results:
        partial += r["op"].astype(np.float64)
    out = (partial + b_proj.astype(np.float64)).astype(np.float32)
    return out.reshape(B, T, C)


# revision 22
# speedup vs baseline: 1.0047x; 1.0047x over previous
"""Trainium2 Bass kernel for causal self-attention (B=2, T=2048, C=1024, H=16).

Sharding: tensor-parallel over heads. Each of the 8 cores owns 2 heads:
  - qkv weight column slices (128 q cols, 128 k cols, 128 v cols),
  - the full attention for its (batch, head) pairs,
  - a row-slice (128 rows) of w_proj -> partial [B*T, C] output.
Host side: x is transposed/cast to bf16 once (shared by all cores); the 8
partial proj outputs are summed + b_proj to form the final output.

Per-core device kernel (all matmuls bf16, fp32 accumulate):
  1. qkv^T projection: Q^T,K^T produced with head-dim on partitions
     ([128 = 2 heads x 64, T]); V produced in natural [t, v] layout with an
     appended ones column so the softmax denominator falls out of the AV
     matmul as a 65th output row.
  2. Causal attention over 128-wide k chunks x 512-wide q tiles computing
     S^T = [k, q] blocks for BOTH heads into one 2-bank PSUM tile
     [128, 2, 512] (PE, two matmuls on disjoint array row groups), one
     fused exp(scale*s) PSUM->SBUF bf16 per chunk (ACT), bf16 0/1
     triangle-mask multiply on the diagonal block (DVE), AV accumulation
     with the ones-row denominator (PE).
  3. Normalize: fast reciprocal of the denominator row (DVE),
     partition-broadcast (GPSIMD), multiply (DVE) -> attn_out^T bf16.
  4. proj emitted one q-tile late (PE never head-of-line blocks on the
     normalize chain); all phases share one PSUM pool so there are no
     pool-close barriers (a multi-us PE gap would re-throttle the PE clock
     to 1.2GHz via the HAM activity monitor).
"""

import numpy as np
import ml_dtypes
from contextlib import ExitStack

import concourse.bass as bass
import concourse.bacc as bacc
import concourse.mybir as mybir
import concourse.tile as tile

BF16 = mybir.dt.bfloat16
F32 = mybir.dt.float32

B, T, C = 2, 2048, 1024
H = 16
D = C // H  # 64
SCALE = D ** -0.5
N_CORES = 8
HEADS_PER_CORE = H // N_CORES  # 2
CL = HEADS_PER_CORE * D  # 128 local channels
P = 128
QT = 512  # q tile width (free dim of S^T blocks)
KC = 128  # k chunk (partition dim of S^T blocks)


def build_nc(t_batch=T, n_batch=B):
    """Build the per-core Bass module. t_batch/n_batch shrinkable for sim."""
    tt = t_batch * n_batch
    nqt = t_batch // QT  # q tiles per batch
    ntt = tt // QT       # t tiles for the qkv projection phase
    ntc = tt // P        # 128-wide t chunks
    cdim = C
    nco = cdim // P      # contraction chunks for qkv projection

    nc = bacc.Bacc("TRN2", target_bir_lowering=False)
    xt = nc.dram_tensor("xt", [cdim, tt], BF16, kind="ExternalInput")
    wq = nc.dram_tensor("wq", [cdim, 3 * CL], BF16, kind="ExternalInput")
    bq = nc.dram_tensor("bq", [3 * CL], F32, kind="ExternalInput")
    bv = nc.dram_tensor("bv", [P, CL], F32, kind="ExternalInput")
    wp = nc.dram_tensor("wp", [CL, C], BF16, kind="ExternalInput")
    mk = nc.dram_tensor("mk", [4 * P, QT], BF16, kind="ExternalInput")
    op = nc.dram_tensor("op", [tt, C], F32, kind="ExternalOutput")

    with tile.TileContext(nc) as tc, ExitStack() as ctx:
        singles = ctx.enter_context(tc.tile_pool(name="singles", bufs=1))
        # one shared PSUM pool for all phases: "s" = 2-bank [128, 2, 512]
        # S^T blocks (also qkv q/k psums and proj psums), o0/o1 = AV accums.
        psum = ctx.enter_context(tc.tile_pool(name="psum", bufs=2, space="PSUM"))
        xt_pool = ctx.enter_context(tc.tile_pool(name="xt_pool", bufs=2))
        pt_pool = ctx.enter_context(tc.tile_pool(name="pt_pool", bufs=3))
        nrm_pool = ctx.enter_context(tc.tile_pool(name="nrm_pool", bufs=2))
        ob_pool = ctx.enter_context(tc.tile_pool(name="ob_pool", bufs=3))

        qkT = singles.tile([P, 2, tt], BF16)  # [:,0,:]=Q^T  [:,1,:]=K^T
        vsb = singles.tile([P, ntc, 2 * (D + 1)], BF16)  # V_ext, both heads
        aoT = singles.tile([P, tt], BF16)  # normalized attn out^T
        wq_sb = singles.tile([P, nco, 3 * CL], BF16)
        bq_sb = singles.tile([P, 3], F32)
        bv_sb = singles.tile([P, CL], F32)
        wp_sb = singles.tile([CL, C], BF16)
        mask_sb = singles.tile([P, 4, QT], BF16)

        # Preloads go through the single SWDGE queue so downstream consumers
        # only inherit one DMA-semaphore wait.
        nc.gpsimd.dma_start(wq_sb, wq.rearrange("(co p) m -> p co m", p=P))
        nc.gpsimd.dma_start(bq_sb, bq.rearrange("(m p) -> p m", p=P))
        nc.gpsimd.dma_start(bv_sb, bv[:, :])
        nc.gpsimd.dma_start(wp_sb, wp[:, :])
        nc.gpsimd.dma_start(mask_sb, mk.rearrange("(mm p) q -> p mm q", p=P))

        # ---- Phase 1: qkv projection ----
        xt_r = xt.rearrange("(co p) t -> p co t", p=P)
        for j in range(ntt):
            ts = slice(j * QT, (j + 1) * QT)
            xt_t = xt_pool.tile([P, nco, QT], BF16, tag="xt")
            nc.sync.dma_start(xt_t, xt_r[:, :, ts])
            # Q^T and K^T: W chunk stationary, x^T moving; both halves of
            # one 2-bank psum tile, single fused bias-add copy-out.
            ps = psum.tile([P, 2, QT], F32, tag="s", name="ps_qk")
            for mi in range(2):
                for co in range(nco):
                    nc.tensor.matmul(
                        ps[:, mi, :],
                        lhsT=wq_sb[:, co, mi * CL:(mi + 1) * CL],
                        rhs=xt_t[:, co, :],
                        start=(co == 0), stop=(co == nco - 1),
                    )
            nc.vector.tensor_tensor(
                qkT[:, :, ts], ps,
                bq_sb[:, 0:2, None].to_broadcast((P, 2, QT)),
                op=mybir.AluOpType.add)
            # V natural: x^T chunk stationary, W_v moving
            for t2 in range(QT // P):
                tg = j * (QT // P) + t2
                psv = psum.tile([P, CL], F32, tag=f"o{t2 % 2}", name="ps_v")
                for co in range(nco):
                    nc.tensor.matmul(
                        psv,
                        lhsT=xt_t[:, co, t2 * P:(t2 + 1) * P],
                        rhs=wq_sb[:, co, 2 * CL:3 * CL],
                        start=(co == 0), stop=(co == nco - 1),
                    )
                for h in range(2):
                    o0 = h * (D + 1)
                    nc.vector.tensor_add(
                        vsb[:, tg, o0:o0 + D],
                        psv[:, h * D:(h + 1) * D], bv_sb[:, h * D:(h + 1) * D])
                    nc.vector.memset(vsb[:, tg, o0 + D:o0 + D + 1], 1.0)

        # ---- Phase 2+3: causal attention fused with output projection ----
        def emit_proj(base, j):
            # projection for q-tile j (emitted one q-tile late so the PE
            # never head-of-line blocks on the normalize chain)
            for t2 in range(QT // P):
                tg = (base + j * QT) // P + t2
                lhs = aoT[:, tg * P:(tg + 1) * P]
                pp = psum.tile([P, 2, QT], F32, tag="s", name="pp")
                for n in range(C // QT):
                    nc.tensor.matmul(pp[:, n, :], lhsT=lhs,
                                     rhs=wp_sb[:, n * QT:(n + 1) * QT],
                                     start=True, stop=True)
                ob = ob_pool.tile([P, C], F32, tag="ob")
                nc.vector.tensor_copy(ob, pp)
                nc.sync.dma_start(op[tg * P:(tg + 1) * P, :], ob)

        tile_order = [(b * t_batch, j) for b in range(n_batch)
                      for j in range(nqt)]
        pending = None
        for base, j in tile_order:
            if True:
                qs = slice(base + j * QT, base + (j + 1) * QT)
                os_ = [psum.tile([D + 1, QT], F32, tag=f"o{h}", name=f"o{h}")
                       for h in range(2)]
                nch = (j + 1) * (QT // KC)
                for m in range(nch):
                    ks = slice(base + m * KC, base + m * KC + KC)
                    mm = m - j * (QT // KC)
                    s = psum.tile([P, 2, QT], F32, tag="s", name="s")
                    for h in range(2):
                        hp = slice(64 * h, 64 * h + 64)
                        nc.tensor.matmul(s[:, h, :], lhsT=qkT[hp, 1, ks],
                                         rhs=qkT[hp, 0, qs],
                                         start=True, stop=True)
                    pt = pt_pool.tile([P, 2, QT], BF16, tag="pt")
                    if mm >= 0:
                        dcol = mm * KC  # diagonal block column offset
                        if dcol > 0:
                            nc.vector.memset(pt[:, :, 0:dcol], 0.0)
                        nc.scalar.activation(
                            pt[:, :, dcol:], s[:, :, dcol:],
                            mybir.ActivationFunctionType.Exp,
                            bias=0.0, scale=SCALE)
                        # zero the k>q triangle: bf16 0/1 multiply (SBUF)
                        for h in range(2):
                            nc.vector.tensor_mul(
                                pt[:, h, dcol:dcol + KC],
                                pt[:, h, dcol:dcol + KC],
                                mask_sb[:, mm, dcol:dcol + KC])
                    else:
                        nc.scalar.activation(
                            pt, s, mybir.ActivationFunctionType.Exp,
                            bias=0.0, scale=SCALE)
                    tg = (base + m * KC) // P
                    for h in range(2):
                        nc.tensor.matmul(
                            os_[h],
                            lhsT=vsb[:, tg, h * (D + 1):(h + 1) * (D + 1)],
                            rhs=pt[:, h, :],
                            start=(m == 0), stop=(m == nch - 1),
                        )
                # normalize: fast recip of denom row -> bcast (gpsimd) -> mul
                for h in range(2):
                    rec = nrm_pool.tile([1, QT], F32, tag="rec")
                    nc.vector.reciprocal_approx_fast(rec, os_[h][D:D + 1, :])
                    rbc = nrm_pool.tile([D, QT], F32, tag="rbc")
                    nc.gpsimd.partition_broadcast(rbc, rec)
                    nc.vector.tensor_mul(aoT[64 * h:64 * h + 64, qs],
                                         os_[h][0:D, :], rbc)
                if pending is not None:
                    emit_proj(*pending)
                pending = (base, j)
        if pending is not None:
            emit_proj(*pending)

    nc.finalize()
    return nc


def make_masks():
    """[4*128, 512] bf16 0/1 keep-masks: row kl of block mm, col c is kept
    iff global k (= 128*mm + kl) <= q (= c)."""
    kl = np.arange(P)[:, None]
    c = np.arange(QT)[None, :]
    out = np.zeros((4 * P, QT), ml_dtypes.bfloat16)
    for mm in range(4):
        out[mm * P:(mm + 1) * P] = (mm * P + kl <= c).astype(ml_dtypes.bfloat16)
    return out


def make_in_maps(x, w_qkv, b_qkv, w_proj, t_batch=T, n_batch=B):
    bf = ml_dtypes.bfloat16
    tt = t_batch * n_batch
    x2 = np.ascontiguousarray(x.reshape(tt, C))
    xt = np.ascontiguousarray(x2.T).astype(bf)
    masks = make_masks()
    in_maps = []
    for i in range(N_CORES):
        cs = slice(CL * i, CL * (i + 1))
        wq_c = np.concatenate(
            [w_qkv[:, cs], w_qkv[:, C + CL * i:C + CL * (i + 1)],
             w_qkv[:, 2 * C + CL * i:2 * C + CL * (i + 1)]], axis=1).astype(bf)
        bq_c = np.concatenate(
            [b_qkv[cs], b_qkv[C + CL * i:C + CL * (i + 1)],
             b_qkv[2 * C + CL * i:2 * C + CL * (i + 1)]]).astype(np.float32)
        bv_c = np.ascontiguousarray(np.broadcast_to(
            b_qkv[2 * C + CL * i:2 * C + CL * (i + 1)][None, :],
            (P, CL))).astype(np.float32)
        wp_c = np.ascontiguousarray(w_proj[cs, :]).astype(bf)
        in_maps.append({
            "xt": xt, "wq": np.ascontiguousarray(wq_c), "bq": bq_c,
            "bv": bv_c, "wp": wp_c, "mk": masks,
        })
    return in_maps


_CACHE = {}


def kernel(x, w_qkv, b_qkv, w_proj, b_proj):
    from concourse.bass_utils import run_bass_kernel_spmd

    x = np.asarray(x, np.float32)
    w_qkv = np.asarray(w_qkv, np.float32)
    b_qkv = np.asarray(b_qkv, np.float32)
    w_proj = np.asarray(w_proj, np.float32)
    b_proj = np.asarray(b_proj, np.float32)

    if "nc" not in _CACHE:
        _CACHE["nc"] = build_nc()
    nc = _CACHE["nc"]
    in_maps = make_in_maps(x, w_qkv, b_qkv, w_proj)
    res = run_bass_kernel_spmd(nc, in_maps, core_ids=list(range(N_CORES)))
    partial = np.zeros((B * T, C), np.float64)
    for r in res.results:
        partial += r["op"].astype(np.float64)
    out = (partial + b_proj.astype(np.float64)).astype(np.float32)
    return out.reshape(B, T, C)


# revision 23
# speedup vs baseline: 1.1178x; 1.1126x over previous
"""Trainium2 Bass kernel for causal self-attention (B=2, T=2048, C=1024, H=16).

Sharding: tensor-parallel over heads. Each of the 8 cores owns 2 heads:
  - qkv weight column slices (128 q cols, 128 k cols, 128 v cols),
  - the full attention for its (batch, head) pairs,
  - a row-slice (128 rows) of w_proj -> partial [B*T, C] output.
Host side: x is transposed/cast to bf16 once (shared by all cores); the 8
partial proj outputs are summed + b_proj to form the final output.

Per-core device kernel (all matmuls bf16, fp32 accumulate):
  1. qkv^T projection: Q^T,K^T produced with head-dim on partitions
     ([128 = 2 heads x 64, T]); V produced in natural [t, v] layout with an
     appended ones column so the softmax denominator falls out of the AV
     matmul as a 65th output row.
  2. Causal attention over 128-wide k chunks x 512-wide q tiles computing
     S^T = [k, q] blocks for BOTH heads into one 2-bank PSUM tile
     [128, 2, 512] (PE, two matmuls on disjoint array row groups), one
     fused exp(scale*s) PSUM->SBUF bf16 per chunk (ACT), bf16 0/1
     triangle-mask multiply on the diagonal block (DVE), AV accumulation
     with the ones-row denominator (PE).
  3. Normalize: fast reciprocal of the denominator row (DVE),
     partition-broadcast (GPSIMD), multiply (DVE) -> attn_out^T bf16.
  4. proj emitted one q-tile late (PE never head-of-line blocks on the
     normalize chain); all phases share one PSUM pool so there are no
     pool-close barriers (a multi-us PE gap would re-throttle the PE clock
     to 1.2GHz via the HAM activity monitor).
"""

import numpy as np
import ml_dtypes
from contextlib import ExitStack

import concourse.bass as bass
import concourse.bacc as bacc
import concourse.mybir as mybir
import concourse.tile as tile

BF16 = mybir.dt.bfloat16
F32 = mybir.dt.float32

B, T, C = 2, 2048, 1024
H = 16
D = C // H  # 64
SCALE = D ** -0.5
N_CORES = 8
HEADS_PER_CORE = H // N_CORES  # 2
CL = HEADS_PER_CORE * D  # 128 local channels
P = 128
QT = 512  # q tile width (free dim of S^T blocks)
KC = 128  # k chunk (partition dim of S^T blocks)


def build_nc(t_batch=T, n_batch=B):
    """Build the per-core Bass module. t_batch/n_batch shrinkable for sim."""
    tt = t_batch * n_batch
    nqt = t_batch // QT  # q tiles per batch
    ntt = tt // QT       # t tiles for the qkv projection phase
    ntc = tt // P        # 128-wide t chunks
    cdim = C
    nco = cdim // P      # contraction chunks for qkv projection

    nc = bacc.Bacc("TRN2", target_bir_lowering=False)
    xt = nc.dram_tensor("xt", [cdim, tt], BF16, kind="ExternalInput")
    wq = nc.dram_tensor("wq", [cdim, 3 * CL], BF16, kind="ExternalInput")
    bq = nc.dram_tensor("bq", [3 * CL], F32, kind="ExternalInput")
    bv = nc.dram_tensor("bv", [P, CL], F32, kind="ExternalInput")
    wp = nc.dram_tensor("wp", [CL, C], BF16, kind="ExternalInput")
    mk = nc.dram_tensor("mk", [4 * P, QT], BF16, kind="ExternalInput")
    op = nc.dram_tensor("op", [tt, C], F32, kind="ExternalOutput")

    with tile.TileContext(nc) as tc, ExitStack() as ctx:
        singles = ctx.enter_context(tc.tile_pool(name="singles", bufs=1))
        # one shared PSUM pool for all phases: "s" = 2-bank [128, 2, 512]
        # S^T blocks (also qkv q/k psums and proj psums), o0/o1 = AV accums.
        psum = ctx.enter_context(tc.tile_pool(name="psum", bufs=2, space="PSUM"))
        xt_pool = ctx.enter_context(tc.tile_pool(name="xt_pool", bufs=2))
        pt_pool = ctx.enter_context(tc.tile_pool(name="pt_pool", bufs=3))
        nrm_pool = ctx.enter_context(tc.tile_pool(name="nrm_pool", bufs=2))
        ob_pool = ctx.enter_context(tc.tile_pool(name="ob_pool", bufs=3))

        qkT = singles.tile([P, 2, tt], BF16)  # [:,0,:]=Q^T  [:,1,:]=K^T
        vsb = singles.tile([P, ntc, 2 * (D + 1)], BF16)  # V_ext, both heads
        aoT = singles.tile([P, tt], BF16)  # normalized attn out^T
        wq_sb = singles.tile([P, nco, 3 * CL], BF16)
        bq_sb = singles.tile([P, 3], F32)
        bv_sb = singles.tile([P, CL], F32)
        wp_sb = singles.tile([CL, C], BF16)
        mask_sb = singles.tile([P, 4, QT], BF16)

        # Preloads go through the single SWDGE queue so downstream consumers
        # only inherit one DMA-semaphore wait.
        nc.gpsimd.dma_start(wq_sb, wq.rearrange("(co p) m -> p co m", p=P))
        nc.gpsimd.dma_start(bq_sb, bq.rearrange("(m p) -> p m", p=P))
        nc.gpsimd.dma_start(bv_sb, bv[:, :])
        nc.gpsimd.dma_start(wp_sb, wp[:, :])
        nc.gpsimd.dma_start(mask_sb, mk.rearrange("(mm p) q -> p mm q", p=P))

        # ---- Phase 1: qkv projection ----
        xt_r = xt.rearrange("(co p) t -> p co t", p=P)
        for j in range(ntt):
            ts = slice(j * QT, (j + 1) * QT)
            xt_t = xt_pool.tile([P, nco, QT], BF16, tag="xt")
            nc.sync.dma_start(xt_t, xt_r[:, :, ts])
            # Q^T and K^T: W chunk stationary, x^T moving; both halves of
            # one 2-bank psum tile, single fused bias-add copy-out.
            ps = psum.tile([P, 2, QT], F32, tag="s", name="ps_qk")
            for mi in range(2):
                for co in range(nco):
                    nc.tensor.matmul(
                        ps[:, mi, :],
                        lhsT=wq_sb[:, co, mi * CL:(mi + 1) * CL],
                        rhs=xt_t[:, co, :],
                        start=(co == 0), stop=(co == nco - 1),
                    )
            nc.vector.tensor_tensor(
                qkT[:, :, ts], ps,
                bq_sb[:, 0:2, None].to_broadcast((P, 2, QT)),
                op=mybir.AluOpType.add)
            # V natural: x^T chunk stationary, W_v moving
            for t2 in range(QT // P):
                tg = j * (QT // P) + t2
                psv = psum.tile([P, CL], F32, tag=f"o{t2 % 2}", name="ps_v")
                for co in range(nco):
                    nc.tensor.matmul(
                        psv,
                        lhsT=xt_t[:, co, t2 * P:(t2 + 1) * P],
                        rhs=wq_sb[:, co, 2 * CL:3 * CL],
                        start=(co == 0), stop=(co == nco - 1),
                    )
                for h in range(2):
                    o0 = h * (D + 1)
                    nc.vector.tensor_add(
                        vsb[:, tg, o0:o0 + D],
                        psv[:, h * D:(h + 1) * D], bv_sb[:, h * D:(h + 1) * D])
                    nc.vector.memset(vsb[:, tg, o0 + D:o0 + D + 1], 1.0)

        # ---- Phase 2+3: causal attention fused with output projection ----
        def emit_proj(base, j):
            # projection for q-tile j (emitted one q-tile late so the PE
            # never head-of-line blocks on the normalize chain)
            for t2 in range(QT // P):
                tg = (base + j * QT) // P + t2
                lhs = aoT[:, tg * P:(tg + 1) * P]
                pp = psum.tile([P, 2, QT], F32, tag="s", name="pp")
                for n in range(C // QT):
                    nc.tensor.matmul(pp[:, n, :], lhsT=lhs,
                                     rhs=wp_sb[:, n * QT:(n + 1) * QT],
                                     start=True, stop=True)
                ob = ob_pool.tile([P, C], F32, tag="ob")
                nc.any.tensor_copy(ob, pp)
                nc.sync.dma_start(op[tg * P:(tg + 1) * P, :], ob)

        tile_order = [(b * t_batch, j) for b in range(n_batch)
                      for j in range(nqt)]
        pending = None
        for base, j in tile_order:
            if True:
                qs = slice(base + j * QT, base + (j + 1) * QT)
                os_ = [psum.tile([D + 1, QT], F32, tag=f"o{h}", name=f"o{h}")
                       for h in range(2)]
                nch = (j + 1) * (QT // KC)
                for m in range(nch):
                    ks = slice(base + m * KC, base + m * KC + KC)
                    mm = m - j * (QT // KC)
                    s = psum.tile([P, 2, QT], F32, tag="s", name="s")
                    for h in range(2):
                        hp = slice(64 * h, 64 * h + 64)
                        nc.tensor.matmul(s[:, h, :], lhsT=qkT[hp, 1, ks],
                                         rhs=qkT[hp, 0, qs],
                                         start=True, stop=True)
                    pt = pt_pool.tile([P, 2, QT], BF16, tag="pt")
                    if mm >= 0:
                        dcol = mm * KC  # diagonal block column offset
                        if dcol > 0:
                            nc.vector.memset(pt[:, :, 0:dcol], 0.0)
                        nc.scalar.activation(
                            pt[:, :, dcol:], s[:, :, dcol:],
                            mybir.ActivationFunctionType.Exp,
                            bias=0.0, scale=SCALE)
                        # zero the k>q triangle: bf16 0/1 multiply (SBUF)
                        for h in range(2):
                            nc.vector.tensor_mul(
                                pt[:, h, dcol:dcol + KC],
                                pt[:, h, dcol:dcol + KC],
                                mask_sb[:, mm, dcol:dcol + KC])
                    else:
                        nc.scalar.activation(
                            pt, s, mybir.ActivationFunctionType.Exp,
                            bias=0.0, scale=SCALE)
                    tg = (base + m * KC) // P
                    for h in range(2):
                        nc.tensor.matmul(
                            os_[h],
                            lhsT=vsb[:, tg, h * (D + 1):(h + 1) * (D + 1)],
                            rhs=pt[:, h, :],
                            start=(m == 0), stop=(m == nch - 1),
                        )
                # normalize: fast recip of denom row -> bcast (gpsimd) -> mul
                for h in range(2):
                    rec = nrm_pool.tile([1, QT], F32, tag="rec")
                    nc.vector.reciprocal_approx_fast(rec, os_[h][D:D + 1, :])
                    rbc = nrm_pool.tile([D, QT], F32, tag="rbc")
                    nc.gpsimd.partition_broadcast(rbc, rec)
                    nc.vector.tensor_mul(aoT[64 * h:64 * h + 64, qs],
                                         os_[h][0:D, :], rbc)
                if pending is not None:
                    emit_proj(*pending)
                pending = (base, j)
        if pending is not None:
            emit_proj(*pending)

    nc.finalize()
    return nc


def make_masks():
    """[4*128, 512] bf16 0/1 keep-masks: row kl of block mm, col c is kept
    iff global k (= 128*mm + kl) <= q (= c)."""
    kl = np.arange(P)[:, None]
    c = np.arange(QT)[None, :]
    out = np.zeros((4 * P, QT), ml_dtypes.bfloat16)
    for mm in range(4):
        out[mm * P:(mm + 1) * P] = (mm * P + kl <= c).astype(ml_dtypes.bfloat16)
    return out


def make_in_maps(x, w_qkv, b_qkv, w_proj, t_batch=T, n_batch=B):
    bf = ml_dtypes.bfloat16
    tt = t_batch * n_batch
    x2 = np.ascontiguousarray(x.reshape(tt, C))
    xt = np.ascontiguousarray(x2.T).astype(bf)
    masks = make_masks()
    in_maps = []
    for i in range(N_CORES):
        cs = slice(CL * i, CL * (i + 1))
        wq_c = np.concatenate(
            [w_qkv[:, cs], w_qkv[:, C + CL * i:C + CL * (i + 1)],
             w_qkv[:, 2 * C + CL * i:2 * C + CL * (i + 1)]], axis=1).astype(bf)
        bq_c = np.concatenate(
            [b_qkv[cs], b_qkv[C + CL * i:C + CL * (i + 1)],
             b_qkv[2 * C + CL * i:2 * C + CL * (i + 1)]]).astype(np.float32)
        bv_c = np.ascontiguousarray(np.broadcast_to(
            b_qkv[2 * C + CL * i:2 * C + CL * (i + 1)][None, :],
            (P, CL))).astype(np.float32)
        wp_c = np.ascontiguousarray(w_proj[cs, :]).astype(bf)
        in_maps.append({
            "xt": xt, "wq": np.ascontiguousarray(wq_c), "bq": bq_c,
            "bv": bv_c, "wp": wp_c, "mk": masks,
        })
    return in_maps


_CACHE = {}


def kernel(x, w_qkv, b_qkv, w_proj, b_proj):
    from concourse.bass_utils import run_bass_kernel_spmd

    x = np.asarray(x, np.float32)
    w_qkv = np.asarray(w_qkv, np.float32)
    b_qkv = np.asarray(b_qkv, np.float32)
    w_proj = np.asarray(w_proj, np.float32)
    b_proj = np.asarray(b_proj, np.float32)

    if "nc" not in _CACHE:
        _CACHE["nc"] = build_nc()
    nc = _CACHE["nc"]
    in_maps = make_in_maps(x, w_qkv, b_qkv, w_proj)
    res = run_bass_kernel_spmd(nc, in_maps, core_ids=list(range(N_CORES)))
    partial = np.zeros((B * T, C), np.float64)
    for r in res.results:
        partial += r["op"].astype(np.float64)
    out = (partial + b_proj.astype(np.float64)).astype(np.float32)
    return out.reshape(B, T, C)


# revision 26
# speedup vs baseline: 1.1421x; 1.0218x over previous
"""Trainium2 Bass kernel for causal self-attention (B=2, T=2048, C=1024, H=16).

Sharding: tensor-parallel over heads. Each of the 8 cores owns 2 heads:
  - qkv weight column slices (128 q cols, 128 k cols, 128 v cols),
  - the full attention for its (batch, head) pairs,
  - a row-slice (128 rows) of w_proj -> partial [B*T, C] output.
Host side: x is transposed/cast to bf16 once (shared by all cores); the 8
partial proj outputs are summed + b_proj to form the final output.

Per-core device kernel (all matmuls bf16, fp32 accumulate):
  1. qkv^T projection: Q^T,K^T produced with head-dim on partitions
     ([128 = 2 heads x 64, T]); V produced in natural [t, v] layout with an
     appended ones column so the softmax denominator falls out of the AV
     matmul as a 65th output row.
  2. Causal attention over 128-wide k chunks x 512-wide q tiles computing
     S^T = [k, q] blocks for BOTH heads into one 2-bank PSUM tile
     [128, 2, 512] (PE, two matmuls on disjoint array row groups), one
     fused exp(scale*s) PSUM->SBUF bf16 per chunk (ACT), bf16 0/1
     triangle-mask multiply on the diagonal block (DVE), AV accumulation
     with the ones-row denominator (PE).
  3. Normalize: fast reciprocal of the denominator row (DVE),
     partition-broadcast (GPSIMD), multiply (DVE) -> attn_out^T bf16.
  4. proj emitted one q-tile late (PE never head-of-line blocks on the
     normalize chain); all phases share one PSUM pool so there are no
     pool-close barriers (a multi-us PE gap would re-throttle the PE clock
     to 1.2GHz via the HAM activity monitor).
"""

import numpy as np
import ml_dtypes
from contextlib import ExitStack

import concourse.bass as bass
import concourse.bacc as bacc
import concourse.mybir as mybir
import concourse.tile as tile

BF16 = mybir.dt.bfloat16
F32 = mybir.dt.float32

B, T, C = 2, 2048, 1024
H = 16
D = C // H  # 64
SCALE = D ** -0.5
N_CORES = 8
HEADS_PER_CORE = H // N_CORES  # 2
CL = HEADS_PER_CORE * D  # 128 local channels
P = 128
QT = 512  # q tile width (free dim of S^T blocks)
KC = 128  # k chunk (partition dim of S^T blocks)


def build_nc(t_batch=T, n_batch=B):
    """Build the per-core Bass module. t_batch/n_batch shrinkable for sim."""
    tt = t_batch * n_batch
    nqt = t_batch // QT  # q tiles per batch
    ntt = tt // QT       # t tiles for the qkv projection phase
    ntc = tt // P        # 128-wide t chunks
    cdim = C
    nco = cdim // P      # contraction chunks for qkv projection

    nc = bacc.Bacc("TRN2", target_bir_lowering=False)
    xt = nc.dram_tensor("xt", [cdim, tt], BF16, kind="ExternalInput")
    wq = nc.dram_tensor("wq", [cdim, 3 * CL], BF16, kind="ExternalInput")
    bq = nc.dram_tensor("bq", [3 * CL], F32, kind="ExternalInput")
    bv = nc.dram_tensor("bv", [P, CL], F32, kind="ExternalInput")
    wp = nc.dram_tensor("wp", [CL, C], BF16, kind="ExternalInput")
    mk = nc.dram_tensor("mk", [4 * P, QT], BF16, kind="ExternalInput")
    op = nc.dram_tensor("op", [tt, C], F32, kind="ExternalOutput")

    with tile.TileContext(nc) as tc, ExitStack() as ctx:
        singles = ctx.enter_context(tc.tile_pool(name="singles", bufs=1))
        # one shared PSUM pool for all phases: "s" = 2-bank [128, 2, 512]
        # S^T blocks (also qkv q/k psums and proj psums), o0/o1 = AV accums.
        psum = ctx.enter_context(tc.tile_pool(name="psum", bufs=2, space="PSUM"))
        xt_pool = ctx.enter_context(tc.tile_pool(name="xt_pool", bufs=3))
        pt_pool = ctx.enter_context(tc.tile_pool(name="pt_pool", bufs=4))
        nrm_pool = ctx.enter_context(tc.tile_pool(name="nrm_pool", bufs=2))
        ob_pool = ctx.enter_context(tc.tile_pool(name="ob_pool", bufs=4))

        qkT = singles.tile([P, 2, tt], BF16)  # [:,0,:]=Q^T  [:,1,:]=K^T
        vsb = singles.tile([P, ntc, 2 * (D + 1)], BF16)  # V_ext, both heads
        aoT = singles.tile([P, tt], BF16)  # normalized attn out^T
        wq_sb = singles.tile([P, nco, 3 * CL], BF16)
        bq_sb = singles.tile([P, 3], F32)
        bv_sb = singles.tile([P, CL], F32)
        wp_sb = singles.tile([CL, C], BF16)
        mask_sb = singles.tile([P, 4, QT], BF16)

        # Preloads go through the single SWDGE queue so downstream consumers
        # only inherit one DMA-semaphore wait.
        nc.gpsimd.dma_start(wq_sb, wq.rearrange("(co p) m -> p co m", p=P))
        nc.gpsimd.dma_start(bq_sb, bq.rearrange("(m p) -> p m", p=P))
        nc.gpsimd.dma_start(bv_sb, bv[:, :])
        nc.gpsimd.dma_start(wp_sb, wp[:, :])
        nc.gpsimd.dma_start(mask_sb, mk.rearrange("(mm p) q -> p mm q", p=P))

        # ---- Phase 1: qkv projection ----
        xt_r = xt.rearrange("(co p) t -> p co t", p=P)
        for j in range(ntt):
            ts = slice(j * QT, (j + 1) * QT)
            xt_t = xt_pool.tile([P, nco, QT], BF16, tag="xt")
            for co in range(nco):
                nc.sync.dma_start(xt_t[:, co, :], xt_r[:, co, ts])
            # Q^T and K^T: W chunk stationary, x^T moving; both halves of
            # one 2-bank psum tile, single fused bias-add copy-out.
            ps = psum.tile([P, 2, QT], F32, tag="s", name="ps_qk")
            for mi in range(2):
                for co in range(nco):
                    nc.tensor.matmul(
                        ps[:, mi, :],
                        lhsT=wq_sb[:, co, mi * CL:(mi + 1) * CL],
                        rhs=xt_t[:, co, :],
                        start=(co == 0), stop=(co == nco - 1),
                    )
            nc.vector.tensor_tensor(
                qkT[:, :, ts], ps,
                bq_sb[:, 0:2, None].to_broadcast((P, 2, QT)),
                op=mybir.AluOpType.add)
            # V natural: x^T chunk stationary, W_v moving
            for t2 in range(QT // P):
                tg = j * (QT // P) + t2
                psv = psum.tile([P, CL], F32, tag=f"o{t2 % 2}", name="ps_v")
                for co in range(nco):
                    nc.tensor.matmul(
                        psv,
                        lhsT=xt_t[:, co, t2 * P:(t2 + 1) * P],
                        rhs=wq_sb[:, co, 2 * CL:3 * CL],
                        start=(co == 0), stop=(co == nco - 1),
                    )
                for h in range(2):
                    o0 = h * (D + 1)
                    nc.vector.tensor_add(
                        vsb[:, tg, o0:o0 + D],
                        psv[:, h * D:(h + 1) * D], bv_sb[:, h * D:(h + 1) * D])
                    nc.vector.memset(vsb[:, tg, o0 + D:o0 + D + 1], 1.0)

        # ---- Phase 2+3: causal attention fused with output projection ----
        def emit_proj(base, j):
            # projection for q-tile j (emitted one q-tile late so the PE
            # never head-of-line blocks on the normalize chain)
            for t2 in range(QT // P):
                tg = (base + j * QT) // P + t2
                lhs = aoT[:, tg * P:(tg + 1) * P]
                pp = psum.tile([P, 2, QT], F32, tag="s", name="pp")
                for n in range(C // QT):
                    nc.tensor.matmul(pp[:, n, :], lhsT=lhs,
                                     rhs=wp_sb[:, n * QT:(n + 1) * QT],
                                     start=True, stop=True)
                ob = ob_pool.tile([P, C], F32, tag="ob")
                nc.any.tensor_copy(ob, pp)
                nc.sync.dma_start(op[tg * P:(tg + 1) * P, :], ob)

        # batch 0 ascending (starts as soon as its first qkv tile lands),
        # batch 1 descending so the kernel tail is the smallest q-tile
        tile_order = [(0, j) for j in range(nqt)] + \
                     [((n_batch - 1) * t_batch, j) for j in reversed(range(nqt))]
        pending = None
        for base, j in tile_order:
            if True:
                qs = slice(base + j * QT, base + (j + 1) * QT)
                os_ = [psum.tile([D + 1, QT], F32, tag=f"o{h}", name=f"o{h}")
                       for h in range(2)]
                nch = (j + 1) * (QT // KC)
                for m in range(nch):
                    ks = slice(base + m * KC, base + m * KC + KC)
                    mm = m - j * (QT // KC)
                    s = psum.tile([P, 2, QT], F32, tag="s", name="s")
                    for h in range(2):
                        hp = slice(64 * h, 64 * h + 64)
                        nc.tensor.matmul(s[:, h, :], lhsT=qkT[hp, 1, ks],
                                         rhs=qkT[hp, 0, qs],
                                         start=True, stop=True)
                    pt = pt_pool.tile([P, 2, QT], BF16, tag="pt")
                    if mm >= 0:
                        dcol = mm * KC  # diagonal block column offset
                        if dcol > 0:
                            nc.vector.memset(pt[:, :, 0:dcol], 0.0)
                        nc.scalar.activation(
                            pt[:, :, dcol:], s[:, :, dcol:],
                            mybir.ActivationFunctionType.Exp,
                            bias=0.0, scale=SCALE)
                        # zero the k>q triangle: bf16 0/1 multiply (SBUF)
                        for h in range(2):
                            nc.vector.tensor_mul(
                                pt[:, h, dcol:dcol + KC],
                                pt[:, h, dcol:dcol + KC],
                                mask_sb[:, mm, dcol:dcol + KC])
                    else:
                        nc.scalar.activation(
                            pt, s, mybir.ActivationFunctionType.Exp,
                            bias=0.0, scale=SCALE)
                    tg = (base + m * KC) // P
                    for h in range(2):
                        nc.tensor.matmul(
                            os_[h],
                            lhsT=vsb[:, tg, h * (D + 1):(h + 1) * (D + 1)],
                            rhs=pt[:, h, :],
                            start=(m == 0), stop=(m == nch - 1),
                        )
                # normalize: fast recip of denom row -> bcast (gpsimd) -> mul
                for h in range(2):
                    rec = nrm_pool.tile([1, QT], F32, tag="rec")
                    nc.vector.reciprocal_approx_fast(rec, os_[h][D:D + 1, :])
                    rbc = nrm_pool.tile([D, QT], F32, tag="rbc")
                    nc.gpsimd.partition_broadcast(rbc, rec)
                    nc.vector.tensor_mul(aoT[64 * h:64 * h + 64, qs],
                                         os_[h][0:D, :], rbc)
                if pending is not None:
                    emit_proj(*pending)
                pending = (base, j)
        if pending is not None:
            emit_proj(*pending)

    nc.finalize()
    return nc


def make_masks():
    """[4*128, 512] bf16 0/1 keep-masks: row kl of block mm, col c is kept
    iff global k (= 128*mm + kl) <= q (= c)."""
    kl = np.arange(P)[:, None]
    c = np.arange(QT)[None, :]
    out = np.zeros((4 * P, QT), ml_dtypes.bfloat16)
    for mm in range(4):
        out[mm * P:(mm + 1) * P] = (mm * P + kl <= c).astype(ml_dtypes.bfloat16)
    return out


def make_in_maps(x, w_qkv, b_qkv, w_proj, t_batch=T, n_batch=B):
    bf = ml_dtypes.bfloat16
    tt = t_batch * n_batch
    x2 = np.ascontiguousarray(x.reshape(tt, C))
    xt = np.ascontiguousarray(x2.T).astype(bf)
    masks = make_masks()
    in_maps = []
    for i in range(N_CORES):
        cs = slice(CL * i, CL * (i + 1))
        wq_c = np.concatenate(
            [w_qkv[:, cs], w_qkv[:, C + CL * i:C + CL * (i + 1)],
             w_qkv[:, 2 * C + CL * i:2 * C + CL * (i + 1)]], axis=1).astype(bf)
        bq_c = np.concatenate(
            [b_qkv[cs], b_qkv[C + CL * i:C + CL * (i + 1)],
             b_qkv[2 * C + CL * i:2 * C + CL * (i + 1)]]).astype(np.float32)
        bv_c = np.ascontiguousarray(np.broadcast_to(
            b_qkv[2 * C + CL * i:2 * C + CL * (i + 1)][None, :],
            (P, CL))).astype(np.float32)
        wp_c = np.ascontiguousarray(w_proj[cs, :]).astype(bf)
        in_maps.append({
            "xt": xt, "wq": np.ascontiguousarray(wq_c), "bq": bq_c,
            "bv": bv_c, "wp": wp_c, "mk": masks,
        })
    return in_maps


_CACHE = {}


def kernel(x, w_qkv, b_qkv, w_proj, b_proj):
    from concourse.bass_utils import run_bass_kernel_spmd

    x = np.asarray(x, np.float32)
    w_qkv = np.asarray(w_qkv, np.float32)
    b_qkv = np.asarray(b_qkv, np.float32)
    w_proj = np.asarray(w_proj, np.float32)
    b_proj = np.asarray(b_proj, np.float32)

    if "nc" not in _CACHE:
        _CACHE["nc"] = build_nc()
    nc = _CACHE["nc"]
    in_maps = make_in_maps(x, w_qkv, b_qkv, w_proj)
    res = run_bass_kernel_spmd(nc, in_maps, core_ids=list(range(N_CORES)))
    partial = np.zeros((B * T, C), np.float64)
    for r in res.results:
        partial += r["op"].astype(np.float64)
    out = (partial + b_proj.astype(np.float64)).astype(np.float32)
    return out.reshape(B, T, C)


# revision 29
# speedup vs baseline: 1.1626x; 1.0179x over previous
"""Trainium2 Bass kernel for causal self-attention (B=2, T=2048, C=1024, H=16).

Sharding: tensor-parallel over heads. Each of the 8 cores owns 2 heads:
  - qkv weight column slices (128 q cols, 128 k cols, 128 v cols),
  - the full attention for its (batch, head) pairs,
  - a row-slice (128 rows) of w_proj -> partial [B*T, C] output.
Host side: x is transposed/cast to bf16 once (shared by all cores); the 8
partial proj outputs are summed + b_proj to form the final output.

Per-core device kernel (all matmuls bf16, fp32 accumulate):
  1. qkv^T projection: Q^T,K^T produced with head-dim on partitions
     ([128 = 2 heads x 64, T]); V produced in natural [t, v] layout with an
     appended ones column so the softmax denominator falls out of the AV
     matmul as a 65th output row.
  2. Causal attention over 128-wide k chunks x 512-wide q tiles computing
     S^T = [k, q] blocks for BOTH heads into one 2-bank PSUM tile
     [128, 2, 512] (PE, two matmuls on disjoint array row groups), one
     fused exp(scale*s) PSUM->SBUF bf16 per chunk (ACT), bf16 0/1
     triangle-mask multiply on the diagonal block (DVE), AV accumulation
     with the ones-row denominator (PE).
  3. Normalize: fast reciprocal of the denominator row (DVE),
     partition-broadcast (GPSIMD), multiply (DVE) -> attn_out^T bf16.
  4. proj emitted one q-tile late (PE never head-of-line blocks on the
     normalize chain); all phases share one PSUM pool so there are no
     pool-close barriers (a multi-us PE gap would re-throttle the PE clock
     to 1.2GHz via the HAM activity monitor).
"""

import numpy as np
import ml_dtypes
from contextlib import ExitStack

import concourse.bass as bass
import concourse.bacc as bacc
import concourse.mybir as mybir
import concourse.tile as tile

BF16 = mybir.dt.bfloat16
F32 = mybir.dt.float32

B, T, C = 2, 2048, 1024
H = 16
D = C // H  # 64
SCALE = D ** -0.5
N_CORES = 8
HEADS_PER_CORE = H // N_CORES  # 2
CL = HEADS_PER_CORE * D  # 128 local channels
P = 128
QT = 512  # q tile width (free dim of S^T blocks)
KC = 128  # k chunk (partition dim of S^T blocks)


def build_nc(t_batch=T, n_batch=B):
    """Build the per-core Bass module. t_batch/n_batch shrinkable for sim."""
    tt = t_batch * n_batch
    nqt = t_batch // QT  # q tiles per batch
    ntt = tt // QT       # t tiles for the qkv projection phase
    ntc = tt // P        # 128-wide t chunks
    cdim = C
    nco = cdim // P      # contraction chunks for qkv projection

    nc = bacc.Bacc("TRN2", target_bir_lowering=False)
    xt = nc.dram_tensor("xt", [cdim, tt], BF16, kind="ExternalInput")
    wq = nc.dram_tensor("wq", [cdim, 3 * CL], BF16, kind="ExternalInput")
    bq = nc.dram_tensor("bq", [3 * CL], F32, kind="ExternalInput")
    bv = nc.dram_tensor("bv", [P, CL], F32, kind="ExternalInput")
    wp = nc.dram_tensor("wp", [CL, C], BF16, kind="ExternalInput")
    mk = nc.dram_tensor("mk", [4 * P, QT], BF16, kind="ExternalInput")
    op = nc.dram_tensor("op", [tt, C], F32, kind="ExternalOutput")

    with tile.TileContext(nc) as tc, ExitStack() as ctx:
        singles = ctx.enter_context(tc.tile_pool(name="singles", bufs=1))
        # one shared PSUM pool for all phases: "s" = 2-bank [128, 2, 512]
        # S^T blocks (also qkv q/k psums and proj psums), o0/o1 = AV accums.
        psum = ctx.enter_context(tc.tile_pool(name="psum", bufs=2, space="PSUM"))
        xt_pool = ctx.enter_context(tc.tile_pool(name="xt_pool", bufs=3))
        pt_pool = ctx.enter_context(tc.tile_pool(name="pt_pool", bufs=4))
        nrm_pool = ctx.enter_context(tc.tile_pool(name="nrm_pool", bufs=2))
        ob_pool = ctx.enter_context(tc.tile_pool(name="ob_pool", bufs=4))

        qkT = singles.tile([P, 2, tt], BF16)  # [:,0,:]=Q^T  [:,1,:]=K^T
        vsb = singles.tile([P, ntc, 2 * (D + 1)], BF16)  # V_ext, both heads
        aoT = singles.tile([P, tt], BF16)  # normalized attn out^T
        wq_sb = singles.tile([P, nco, 3 * CL], BF16)
        bq_sb = singles.tile([P, 3], F32)
        bv_sb = singles.tile([P, CL], F32)
        wp_sb = singles.tile([CL, C], BF16)
        mask_sb = singles.tile([P, 4, QT], BF16)

        # Preloads go through the single SWDGE queue so downstream consumers
        # only inherit one DMA-semaphore wait.
        nc.gpsimd.dma_start(wq_sb, wq.rearrange("(co p) m -> p co m", p=P))
        nc.gpsimd.dma_start(bq_sb, bq.rearrange("(m p) -> p m", p=P))
        nc.gpsimd.dma_start(bv_sb, bv[:, :])
        nc.gpsimd.dma_start(wp_sb, wp[:, :])
        nc.gpsimd.dma_start(mask_sb, mk.rearrange("(mm p) q -> p mm q", p=P))

        # ---- Phase 1: qkv projection (emitted interleaved, see plan) ----
        xt_r = xt.rearrange("(co p) t -> p co t", p=P)

        def emit_qkv(j):
            ts = slice(j * QT, (j + 1) * QT)
            xt_t = xt_pool.tile([P, nco, QT], BF16, tag="xt")
            for co in range(nco):
                nc.sync.dma_start(xt_t[:, co, :], xt_r[:, co, ts])
            # Q^T and K^T: W chunk stationary, x^T moving; both halves of
            # one 2-bank psum tile, single fused bias-add copy-out.
            ps = psum.tile([P, 2, QT], F32, tag="s", name="ps_qk")
            for mi in range(2):
                for co in range(nco):
                    nc.tensor.matmul(
                        ps[:, mi, :],
                        lhsT=wq_sb[:, co, mi * CL:(mi + 1) * CL],
                        rhs=xt_t[:, co, :],
                        start=(co == 0), stop=(co == nco - 1),
                    )
            nc.vector.tensor_tensor(
                qkT[:, :, ts], ps,
                bq_sb[:, 0:2, None].to_broadcast((P, 2, QT)),
                op=mybir.AluOpType.add)
            # V natural: x^T chunk stationary, W_v moving
            for t2 in range(QT // P):
                tg = j * (QT // P) + t2
                psv = psum.tile([P, CL], F32, tag=f"o{t2 % 2}", name="ps_v")
                for co in range(nco):
                    nc.tensor.matmul(
                        psv,
                        lhsT=xt_t[:, co, t2 * P:(t2 + 1) * P],
                        rhs=wq_sb[:, co, 2 * CL:3 * CL],
                        start=(co == 0), stop=(co == nco - 1),
                    )
                for h in range(2):
                    o0 = h * (D + 1)
                    nc.vector.tensor_add(
                        vsb[:, tg, o0:o0 + D],
                        psv[:, h * D:(h + 1) * D], bv_sb[:, h * D:(h + 1) * D])
                    nc.vector.memset(vsb[:, tg, o0 + D:o0 + D + 1], 1.0)

        # ---- Phase 2+3: causal attention fused with output projection ----
        def emit_proj(base, j):
            # projection for q-tile j (emitted one q-tile late so the PE
            # never head-of-line blocks on the normalize chain)
            for t2 in range(QT // P):
                tg = (base + j * QT) // P + t2
                lhs = aoT[:, tg * P:(tg + 1) * P]
                pp = psum.tile([P, 2, QT], F32, tag="s", name="pp")
                for n in range(C // QT):
                    nc.tensor.matmul(pp[:, n, :], lhsT=lhs,
                                     rhs=wp_sb[:, n * QT:(n + 1) * QT],
                                     start=True, stop=True)
                ob = ob_pool.tile([P, C], F32, tag="ob")
                nc.any.tensor_copy(ob, pp)
                nc.sync.dma_start(op[tg * P:(tg + 1) * P, :], ob)

        pending = None

        def emit_att(base, j):
            nonlocal pending
            if True:
                qs = slice(base + j * QT, base + (j + 1) * QT)
                os_ = [psum.tile([D + 1, QT], F32, tag=f"o{h}", name=f"o{h}")
                       for h in range(2)]
                nch = (j + 1) * (QT // KC)
                for m in range(nch):
                    ks = slice(base + m * KC, base + m * KC + KC)
                    mm = m - j * (QT // KC)
                    s = psum.tile([P, 2, QT], F32, tag="s", name="s")
                    for h in range(2):
                        hp = slice(64 * h, 64 * h + 64)
                        nc.tensor.matmul(s[:, h, :], lhsT=qkT[hp, 1, ks],
                                         rhs=qkT[hp, 0, qs],
                                         start=True, stop=True)
                    pt = pt_pool.tile([P, 2, QT], BF16, tag="pt")
                    if mm >= 0:
                        dcol = mm * KC  # diagonal block column offset
                        if dcol > 0:
                            nc.vector.memset(pt[:, :, 0:dcol], 0.0)
                        nc.scalar.activation(
                            pt[:, :, dcol:], s[:, :, dcol:],
                            mybir.ActivationFunctionType.Exp,
                            bias=0.0, scale=SCALE)
                        # zero the k>q triangle: bf16 0/1 multiply (SBUF)
                        for h in range(2):
                            nc.vector.tensor_mul(
                                pt[:, h, dcol:dcol + KC],
                                pt[:, h, dcol:dcol + KC],
                                mask_sb[:, mm, dcol:dcol + KC])
                    else:
                        nc.scalar.activation(
                            pt, s, mybir.ActivationFunctionType.Exp,
                            bias=0.0, scale=SCALE)
                    tg = (base + m * KC) // P
                    for h in range(2):
                        nc.tensor.matmul(
                            os_[h],
                            lhsT=vsb[:, tg, h * (D + 1):(h + 1) * (D + 1)],
                            rhs=pt[:, h, :],
                            start=(m == 0), stop=(m == nch - 1),
                        )
                # normalize: fast recip of denom row -> bcast (gpsimd) -> mul
                for h in range(2):
                    rec = nrm_pool.tile([1, QT], F32, tag="rec")
                    nc.vector.reciprocal_approx_fast(rec, os_[h][D:D + 1, :])
                    rbc = nrm_pool.tile([D, QT], F32, tag="rbc")
                    nc.gpsimd.partition_broadcast(rbc, rec)
                    nc.vector.tensor_mul(aoT[64 * h:64 * h + 64, qs],
                                         os_[h][0:D, :], rbc)
                if pending is not None:
                    emit_proj(*pending)
                pending = (base, j)

        # Emission plan: batch-0 attention q-tiles interleaved right after
        # the qkv tile that completes their dependencies (ACT starts exp'ing
        # ~10us in instead of idling through the whole qkv phase); batch-1
        # attention after all qkv, in descending size so the kernel tail is
        # the smallest q-tile.
        for j in range(ntt):
            emit_qkv(j)
            if j < nqt:
                emit_att(0, j)
        for j in reversed(range(nqt)):
            emit_att((n_batch - 1) * t_batch, j)
        if pending is not None:
            emit_proj(*pending)

    nc.finalize()
    return nc


def make_masks():
    """[4*128, 512] bf16 0/1 keep-masks: row kl of block mm, col c is kept
    iff global k (= 128*mm + kl) <= q (= c)."""
    kl = np.arange(P)[:, None]
    c = np.arange(QT)[None, :]
    out = np.zeros((4 * P, QT), ml_dtypes.bfloat16)
    for mm in range(4):
        out[mm * P:(mm + 1) * P] = (mm * P + kl <= c).astype(ml_dtypes.bfloat16)
    return out


def make_in_maps(x, w_qkv, b_qkv, w_proj, t_batch=T, n_batch=B):
    bf = ml_dtypes.bfloat16
    tt = t_batch * n_batch
    x2 = np.ascontiguousarray(x.reshape(tt, C))
    xt = np.ascontiguousarray(x2.T).astype(bf)
    masks = make_masks()
    in_maps = []
    for i in range(N_CORES):
        cs = slice(CL * i, CL * (i + 1))
        wq_c = np.concatenate(
            [w_qkv[:, cs], w_qkv[:, C + CL * i:C + CL * (i + 1)],
             w_qkv[:, 2 * C + CL * i:2 * C + CL * (i + 1)]], axis=1).astype(bf)
        bq_c = np.concatenate(
            [b_qkv[cs], b_qkv[C + CL * i:C + CL * (i + 1)],
             b_qkv[2 * C + CL * i:2 * C + CL * (i + 1)]]).astype(np.float32)
        bv_c = np.ascontiguousarray(np.broadcast_to(
            b_qkv[2 * C + CL * i:2 * C + CL * (i + 1)][None, :],
            (P, CL))).astype(np.float32)
        wp_c = np.ascontiguousarray(w_proj[cs, :]).astype(bf)
        in_maps.append({
            "xt": xt, "wq": np.ascontiguousarray(wq_c), "bq": bq_c,
            "bv": bv_c, "wp": wp_c, "mk": masks,
        })
    return in_maps


_CACHE = {}


def kernel(x, w_qkv, b_qkv, w_proj, b_proj):
    from concourse.bass_utils import run_bass_kernel_spmd

    x = np.asarray(x, np.float32)
    w_qkv = np.asarray(w_qkv, np.float32)
    b_qkv = np.asarray(b_qkv, np.float32)
    w_proj = np.asarray(w_proj, np.float32)
    b_proj = np.asarray(b_proj, np.float32)

    if "nc" not in _CACHE:
        _CACHE["nc"] = build_nc()
    nc = _CACHE["nc"]
    in_maps = make_in_maps(x, w_qkv, b_qkv, w_proj)
    res = run_bass_kernel_spmd(nc, in_maps, core_ids=list(range(N_CORES)))
    partial = np.zeros((B * T, C), np.float64)
    for r in res.results:
        partial += r["op"].astype(np.float64)
    out = (partial + b_proj.astype(np.float64)).astype(np.float32)
    return out.reshape(B, T, C)
